# revision 19
# baseline (speedup 1.0000x reference)
"""DGCN diffusion-graph-conv kernel for 8 Trainium2 NeuronCores.

Math (per batch b):
    x_cat = concat(inputs, state_t, ones)      # [N, C+1]  (ones row folds bias)
    out_b = tanh( x_cat @ W0' + sum_s A_s @ (Y1s + 2*A_s @ Y2s) )
  where (projection-first reformulation, exploiting spmm/proj commutation):
    W0'  = W_m0 - W_m2 - W_m4 (+ bias row)     # folds the "-x0" Chebyshev terms
    Y1s  = x_cat @ W_{2s+1},  Y2s = x_cat @ W_{2s+2}     # [N, HID]

Distribution: pure data-parallel over batch (2 batches per core, 8 cores),
no collectives.

Device dataflow (all node-major, zero transposes):
  - projections run with x_cat^T tiles as the PE stationary operand and the
    weight blocks as moving, producing node-major PSUM [128 nodes, 5*HID].
  - A_s is densified on the host into 128x128 bf16 blocks (entries val=1/16,
    exactly representable; duplicate edges accumulated) laid out DMA- and
    LDWEIGHTS-friendly as [ib, j, jb, i].  A_s @ X is then 32 PSUM-accumulated
    matmuls per 128-row tile: lhsT = A^T block (stationary), rhs = X node
    tile [128, 256] (moving), PSUM [128 rows, 256] f32 exact.
  - DVE applies the Chebyshev combines straight out of PSUM.
"""

import numpy as np

import concourse.bass as bass
import concourse.bacc as bacc
import concourse.tile as tile
from concourse import mybir
from concourse.bass import ts
from concourse.bass_utils import run_bass_kernel_spmd

F32 = mybir.dt.float32
BF16 = mybir.dt.bfloat16
Alu = mybir.AluOpType
Act = mybir.ActivationFunctionType

B, N, IN_DIM, HID = 16, 4096, 64, 128
C = IN_DIM + HID              # 192
CB = C + 1                    # +1 ones row (bias folding)
M = 5
DEG = 16
NNZ = N * DEG
N_CORES = 8
BL = B // N_CORES             # 2 batches per core
N_SUP = 2
W2 = BL * HID                 # 256: both batches' features per node
NT = N // 128                 # 32 node tiles

_prog_cache: dict = {}


def _install_ntff_hook():
    """Benchmark-only: wire up the NTFF profile hook that bass_utils
    expects under axon when trace=True (the antenv.axon_hooks shim module
    is absent in this image), and stub out the S3 artifact upload."""
    import sys
    import types

    try:
        import antenv
        import concourse.bass_utils as bu

        bu.upload_artifacts = lambda tmpdir: "local://" + tmpdir
        if "antenv.axon_hooks" in sys.modules:
            return
        import trn_agent_boot.trn_boot as tb

        hook = tb._ntff_profile_via_ctypes("/opt/axon/libaxon_pjrt.so")
        mod = types.ModuleType("antenv.axon_hooks")
        mod.get_axon_ntff_profile_hook = lambda: hook
        mod.set_axon_ntff_profile_hook = lambda h: None
        sys.modules["antenv.axon_hooks"] = mod
        antenv.axon_hooks = mod
    except Exception as e:  # profiling is best-effort
        print(f"ntff hook install failed: {e}")


def _build_program(n_sup: int):
    nc = bacc.Bacc(
        "TRN2",
        target_bir_lowering=False,
        debug=False,
        enable_asserts=False,
        num_devices=N_CORES,
    )

    x0T_d = nc.dram_tensor("x0T", [BL, CB, N], BF16, kind="ExternalInput").ap()
    wc_d = nc.dram_tensor("wc", [CB, M * HID], F32, kind="ExternalInput").ap()
    # A^T blocks, DMA/LDW-friendly: ablk[s, ib, j, jb*128+i] = A_s[ib*128+i,
    # jb*128+j] (val folded in, bf16)
    ablk_d = nc.dram_tensor(
        "ablk", [n_sup, NT, 128, N], BF16, kind="ExternalInput"
    ).ap()
    out_d = nc.dram_tensor("out", [128, NT, W2], F32, kind="ExternalOutput").ap()

    KCH = [(0, 128), (128, CB - 128)]   # C+1 split into partition chunks
    kn1 = CB - 128

    with tile.TileContext(nc) as tc:
        with (
            tc.tile_pool(name="persist", bufs=1) as persist,
            tc.tile_pool(name="big", bufs=4) as bigp,
            tc.tile_pool(name="xstage", bufs=2) as xstage,
            tc.tile_pool(name="psA", bufs=2, space="PSUM") as psA,
            tc.tile_pool(name="psB", bufs=2, space="PSUM") as psB,
            tc.tile_pool(name="psS", bufs=4, space="PSUM") as psS,
        ):
            # ---------- weights ----------
            wst = xstage.tile([128, M * HID], F32, tag="xstage", name="wst0")
            nc.sync.dma_start(out=wst[:], in_=wc_d[0:128, :])
            wc_bf0 = persist.tile([128, M * HID], BF16, tag="wc0")
            nc.scalar.copy(out=wc_bf0[:], in_=wst[:])
            wst2 = xstage.tile([128, M * HID], F32, tag="xstage", name="wst1")
            nc.sync.dma_start(out=wst2[:kn1, :], in_=wc_d[128:CB, :])
            wc_bf1 = persist.tile([128, M * HID], BF16, tag="wc1")
            nc.scalar.copy(out=wc_bf1[:kn1, :], in_=wst2[:kn1, :])
            wc_bf = [wc_bf0, wc_bf1]

            # ---------- load x0T (host pre-cast to bf16) ----------
            # x0T_bf[b]: [128, 8192] bf16; cols [0:4096] = chunk 0 (feats
            # 0..127), cols [4096:8192] = chunk 1 (feats 128..192 on
            # partitions 0..64).  The 16KB slots of pool "big" are later
            # recycled as A-block streaming tiles.
            x0T_bf = []
            for b in range(BL):
                xb = bigp.tile([128, 2 * N], BF16, tag="big", name=f"xb{b}")
                for half in range(2):
                    sl = ts(half, N // 2)
                    nc.sync.dma_start(out=xb[:, sl], in_=x0T_d[b, 0:128, sl])
                    nc.sync.dma_start(
                        out=xb[:kn1, N + half * (N // 2) : N + (half + 1) * (N // 2)],
                        in_=x0T_d[b, 128:CB, sl],
                    )
                x0T_bf.append(xb)

            # ---------- persistent node-major tensors ----------
            y1 = [persist.tile([128, NT, W2], BF16, tag=f"y1_{s}", name=f"y1_{s}")
                  for s in range(n_sup)]
            y2 = [persist.tile([128, NT, W2], BF16, tag=f"y2_{s}", name=f"y2_{s}")
                  for s in range(n_sup)]
            u_t = persist.tile([128, NT, W2], BF16, tag="u")
            acc = persist.tile([128, NT, W2], F32, tag="acc")

            # ---------- projections ----------
            # per (node-tile, batch): stationary = x_cat^T slice, moving =
            # weight blocks; PSUM out node-major [128, m*HID] split 384+256.
            for t in range(NT):
                for b in range(BL):
                    pa = psA.tile([128, 384], F32, tag="psA")
                    pb = psB.tile([128, 256], F32, tag="psB")
                    for kc, (k0, kn) in enumerate(KCH):
                        lhs = x0T_bf[b][:kn, kc * N + t * 128 : kc * N + (t + 1) * 128]
                        nc.tensor.matmul(
                            pa[:], lhsT=lhs, rhs=wc_bf[kc][:kn, 0:384],
                            start=(kc == 0), stop=(kc == 1),
                        )
                        nc.tensor.matmul(
                            pb[:], lhsT=lhs, rhs=wc_bf[kc][:kn, 384:640],
                            start=(kc == 0), stop=(kc == 1),
                        )
                    # m: 0 -> acc, 1 -> y1[0], 2 -> y2[0], 3 -> y1[1], 4 -> y2[1]
                    nc.scalar.copy(out=acc[:, t, ts(b, HID)], in_=pa[:, 0:128])
                    nc.vector.tensor_copy(
                        out=y1[0][:, t, ts(b, HID)], in_=pa[:, 128:256]
                    )
                    nc.scalar.copy(out=y2[0][:, t, ts(b, HID)], in_=pa[:, 256:384])
                    if n_sup > 1:
                        nc.vector.tensor_copy(
                            out=y1[1][:, t, ts(b, HID)], in_=pb[:, 0:128]
                        )
                        nc.scalar.copy(
                            out=y2[1][:, t, ts(b, HID)], in_=pb[:, 128:256]
                        )

            # ---------- dense spmm: out_tile(ib) = sum_jb A^T[jb,ib].T @ X[jb]
            def spmm(s: int, src, sink, phase: str):
                for ib in range(NT):
                    at = bigp.tile([128, N], BF16, tag="big", name=f"a_{phase}{ib}")
                    nc.sync.dma_start(out=at[:], in_=ablk_d[s, ib])
                    ps = psS.tile([128, W2], F32, tag="psS")
                    for jb in range(NT):
                        nc.tensor.matmul(
                            ps[:],
                            lhsT=at[:, ts(jb, 128)],
                            rhs=src[:, jb, :],
                            start=(jb == 0),
                            stop=(jb == NT - 1),
                        )
                    sink(ib, ps)

            def mk_sink_u(s):
                def sink_u(ib, ps):
                    # U = Y1 + 2 * (A @ Y2)
                    nc.vector.scalar_tensor_tensor(
                        out=u_t[:, ib, :],
                        in0=ps[:],
                        scalar=2.0,
                        in1=y1[s][:, ib, :],
                        op0=Alu.mult,
                        op1=Alu.add,
                    )
                return sink_u

            def mk_sink_acc(s):
                def sink_acc(ib, ps):
                    nc.vector.tensor_tensor(
                        out=acc[:, ib, :], in0=ps[:], in1=acc[:, ib, :], op=Alu.add
                    )
                return sink_acc

            for s in range(n_sup):
                spmm(s, y2[s], mk_sink_u(s), f"z{s}")
                spmm(s, u_t, mk_sink_acc(s), f"v{s}")

            # ---------- tanh + store ----------
            for t2 in range(8):
                ot = xstage.tile([128, (NT // 8) * W2], F32, tag="xstage")
                nc.scalar.activation(
                    out=ot[:],
                    in_=acc[:, t2 * (NT // 8) : (t2 + 1) * (NT // 8), :],
                    func=Act.Tanh,
                )
                nc.sync.dma_start(
                    out=out_d[:, t2 * (NT // 8) : (t2 + 1) * (NT // 8), :],
                    in_=ot[:],
                )

    nc.compile()
    return nc


def _build_ablk(sup_cols, sup_vals):
    """Densify the supports into PE-friendly bf16 blocks.

    ablk[s, ib, j, jb*128+i] = A_s[ib*128+i, jb*128+j], duplicates added.
    """
    import ml_dtypes

    ablk = np.zeros((N_SUP, NT, 128, N), dtype=np.float32)
    for s in range(N_SUP):
        rows = np.repeat(np.arange(N, dtype=np.int64), DEG)
        cols = sup_cols[s].astype(np.int64)
        vals = sup_vals[s].astype(np.float32)
        ib, i = rows // 128, rows % 128
        jb, j = cols // 128, cols % 128
        np.add.at(ablk[s], (ib, j, jb * 128 + i), vals)
    return ablk.astype(ml_dtypes.bfloat16)


def _prep_core_inputs(inputs, state_t, weights, biases, sup_cols, sup_vals):
    """Host-side sharding: batch-parallel slices + layout prep."""
    import ml_dtypes

    w5 = weights.reshape(C, M, HID)
    wc = np.zeros((CB, M, HID), dtype=np.float32)
    wc[:C, 0] = w5[:, 0] - w5[:, 2] - w5[:, 4]
    wc[C, 0] = biases.astype(np.float32)          # bias via ones row
    for m in range(1, M):
        wc[:C, m] = w5[:, m]
    wc = np.ascontiguousarray(wc.reshape(CB, M * HID))

    ablk = _build_ablk(sup_cols, sup_vals)

    in_maps = []
    for core in range(N_CORES):
        b0 = core * BL
        xcat = np.concatenate(
            [
                inputs[b0 : b0 + BL],
                state_t[b0 : b0 + BL],
                np.ones((BL, N, 1), dtype=np.float32),
            ],
            axis=2,
        )  # [BL, N, CB]
        x0T = np.ascontiguousarray(xcat.transpose(0, 2, 1)).astype(ml_dtypes.bfloat16)
        in_maps.append({"x0T": x0T, "wc": wc, "ablk": ablk})
    return in_maps


def kernel(
    inputs,
    state_t,
    weights,
    biases,
    sup_rows,
    sup_cols,
    sup_vals,
    _bench=None,
):
    inputs = np.asarray(inputs)
    state_t = np.asarray(state_t)
    weights = np.asarray(weights, dtype=np.float32)
    biases = np.asarray(biases, dtype=np.float32)
    sup_rows = np.asarray(sup_rows)
    sup_cols = np.asarray(sup_cols)
    sup_vals = np.asarray(sup_vals)

    # The model family guarantees the canonical fixed-degree row structure:
    # row i owns COO slots [i*DEG, (i+1)*DEG).
    exp_rows = np.repeat(np.arange(N, dtype=sup_rows.dtype), DEG)
    assert all(np.array_equal(sup_rows[s], exp_rows) for s in range(N_SUP))

    if "prog" not in _prog_cache:
        _prog_cache["prog"] = _build_program(N_SUP)
    nc = _prog_cache["prog"]

    in_maps = _prep_core_inputs(
        inputs, state_t, weights, biases, sup_cols, sup_vals
    )
    trace = _bench is not None
    if trace:
        _install_ntff_hook()
    res = run_bass_kernel_spmd(nc, in_maps, list(range(N_CORES)), trace=trace)
    if _bench is not None:
        _bench["exec_time_ns"] = res.exec_time_ns
        _bench["mean_exec_time_ns"] = res.mean_exec_time_ns
        _bench["results"] = res

    out = np.empty((B, N, HID), dtype=np.float32)
    for core in range(N_CORES):
        o = res.results[core]["out"]  # [128, NT, W2]
        for b in range(BL):
            # node n = t*128 + p ; feature = b*HID + h
            out[core * BL + b] = (
                o[:, :, b * HID : (b + 1) * HID].transpose(1, 0, 2).reshape(N, HID)
            )
    return out
